# revision 51
# baseline (speedup 1.0000x reference)
"""TRN2 Bass kernel for nn_MultiHeadAttention (B=4, S=2048, D=1024, H=16, DH=64).

Sharding (8 cores): core c -> batch b = c//2, head-half hh = c%2 (8 heads each).
Host sums the two per-core partial out-projections per batch.

All matmul operands are bf16 (1 cycle/row at any N, vs f32r's N>=256
requirement); accumulation stays f32 in PSUM.  Measured end-to-end rel err
~5.5e-3 vs the f32 reference (gate 2e-2).

Structure (single TileContext, one fluid phase; x resident in SBUF):
  - scores TRANSPOSED sT[sk, sq] = kT.T @ qT (stationary kT [64, 128-sk],
    moving qT [64, 512]); one 1024-wide exp per sk-tile -> at [128sk, 1024sq]
    bf16.
  - PV reoriented: at tile is STATIONARY [sk 128, sq 128-chunk], V_aug
    [sk 128, 65] is MOVING (N=65; full K=M=128) -> ctx accumulates in natural
    [sq, 65] layout, 65th column = softmax denominator on the sq partition.
    This halves PV's PE cycles vs streaming at with N=512 at M=65.
  - normalization: per-partition reciprocal + tensor_scalar_mul on DVE (no
    PE broadcast matmuls); each head pair shares one ctx_nat [sq 128, 1024]
    buffer per qb; a single blockwise XBAR DMA transpose (3D out AP:
    out[p,c,s] = in[s, c*128+p]) flips all 8 chunks into ctxT [128 feats, sq]
    in one DMA op (~620ns HWDGE overhead per op makes batching essential).
  - v-projection computes V_aug [sk, 65-per-head] with a memset-initialized
    ones column (no 4MB DMA); split into head-half units for fine scheduling.
  - out-projection splits into pairs-0/1 partials (SBUF bf16 accumulator,
    emitted early as filler) and pairs-2/3 completion units.
  - a budget-based filler engine interleaves projection/v/out-proj units into
    the attention stream (ACT needs 1038ns/sk-tile vs PE's 644ns) so PE and
    ACT both stay near-continuously busy; emission-order gates enforce that a
    block's pair projections are emitted before its scores (Tile dependency
    tracking is reader-after-writer in emission order).

This walrus build accepts only ONE sync-wait per instruction, so after
TileContext scheduling extra waits are split into single-wait NoOps on the
same engine (legalize_waits).
"""

import sys

if "/opt/trn_rl_repo" not in sys.path:
    sys.path.insert(0, "/opt/trn_rl_repo")

import numpy as np
import ml_dtypes

import concourse.bass as bass
import concourse.mybir as mybir
import concourse.tile as tile
from concourse.bass_utils import run_bass_kernel_spmd

F32 = mybir.dt.float32
BF16 = mybir.dt.bfloat16
EXP = mybir.ActivationFunctionType.Exp

B, S_FULL, D, H = 4, 2048, 1024, 16
DH = 64
NCORES = 8


def legalize_waits(nc, max_waits=1):
    """Split >max_waits sync-waits per instruction into single-wait NoOps on
    the same engine, placed immediately before (per-engine order preserved)."""
    n = 0
    for fn in nc.m.functions:
        for blk in fn.blocks:
            out = []
            for inst in blk.instructions:
                si = inst.sync_info
                if si is not None and len(si.on_wait) > max_waits:
                    waits = list(si.on_wait)
                    for w in waits[:-max_waits]:
                        nop = mybir.InstNoOp(
                            name=f"WSPLIT-{n}", ins=[], outs=[], engine=inst.engine
                        )
                        n += 1
                        nop.sync_info = mybir.SyncInfo(on_wait=[w], on_update=[])
                        out.append(nop)
                    inst.sync_info = mybir.SyncInfo(
                        on_wait=waits[-max_waits:], on_update=list(si.on_update)
                    )
                out.append(inst)
            blk.instructions[:] = out
    return n


def _bcast_ap(src_ap, parts=128):
    """Partition-broadcast a [1, N] AP to [parts, N] via a step-0 dim."""
    return bass.AP(
        tensor=src_ap.tensor,
        offset=src_ap.offset,
        ap=[[0, parts], list(src_ap.ap[-1])],
    )


def build_nc(S=S_FULL, legalize=True):
    NST = S // 128   # sk tiles
    NSB = S // 512   # 512-wide s blocks (projection granularity)
    nc = bass.Bass()
    xT_d = nc.dram_tensor("xt", [D, S], BF16, kind="ExternalInput")
    wq_d = nc.dram_tensor("wq", [128, 4096], BF16, kind="ExternalInput")
    wk_d = nc.dram_tensor("wk", [128, 4096], BF16, kind="ExternalInput")
    wv_d = nc.dram_tensor("wv", [128, 4096], BF16, kind="ExternalInput")
    wo_d = nc.dram_tensor("wo", [128, 4096], BF16, kind="ExternalInput")
    bqk_d = nc.dram_tensor("bqk", [128, 8], F32, kind="ExternalInput")
    bv_d = nc.dram_tensor("bv", [1, 512], F32, kind="ExternalInput")
    bo_d = nc.dram_tensor("bo", [1, 1024], F32, kind="ExternalInput")
    out_d = nc.dram_tensor("out", [S, 1024], BF16, kind="ExternalOutput")

    with tile.TileContext(nc) as tc, nc.allow_low_precision(
        reason="bf16 matmul operands are intentional; f32 accumulate"
    ):
        with tc.tile_pool(name="persist", bufs=1) as pp, \
             tc.tile_pool(name="psS", bufs=2, space="PSUM") as psS, \
             tc.tile_pool(name="psC", bufs=1, space="PSUM") as psC, \
             tc.tile_pool(name="psP", bufs=2, space="PSUM") as psP, \
             tc.tile_pool(name="atp", bufs=13) as atp, \
             tc.tile_pool(name="cn2", bufs=2) as cn2, \
             tc.tile_pool(name="rzp", bufs=4) as rzp, \
             tc.tile_pool(name="oap", bufs=2) as oap, \
             tc.tile_pool(name="otp", bufs=3) as otp:
            xres = pp.tile([128, 8 * S], BF16)          # d-chunk ch at cols ch*S
            qT = pp.tile([128, 4 * S], BF16)
            kT = pp.tile([128, 4 * S], BF16)
            vall = pp.tile([128, NST * 520], BF16)      # per st: 8 heads x 65
            ctxT = pp.tile([128, 4 * S], BF16)
            wq0 = pp.tile([128, 1024], BF16)
            wk0 = pp.tile([128, 1024], BF16)
            wq2 = pp.tile([128, 3072], BF16)
            wk2 = pp.tile([128, 3072], BF16)
            wv = pp.tile([128, 4096], BF16)
            wo = pp.tile([128, 4096], BF16)
            bqk = pp.tile([128, 8], F32)
            bv_b = pp.tile([128, 512], F32)
            bo_b = pp.tile([128, 1024], F32)

            # ---- DMA issue order: first-needed bytes first ----
            def dma_w_pair0(dst, src_d, chs):
                src = src_d[:, chs[0] * 512: chs[-1] * 512 + 512].rearrange(
                    "p (c w) -> p c w", w=512)[:, :, 0:128]
                nc.sync.dma_start(
                    out=dst[:, chs[0] * 128: chs[-1] * 128 + 128].rearrange(
                        "p (c w) -> p c w", w=128),
                    in_=src)

            def dma_x_part(sb, lo, n):
                # d-chunks lo..lo+n-1 of s-block sb in one DMA
                dst = xres[:, lo * S:(lo + n) * S].rearrange(
                    "p (c s) -> p c s", c=n)[:, :, sb * 512:(sb + 1) * 512]
                src = xT_d[lo * 128:(lo + n) * 128,
                           sb * 512:(sb + 1) * 512].rearrange(
                    "(c p) s -> p c s", c=n)
                nc.sync.dma_start(out=dst, in_=src)

            def dma_x_half(sb, lo):
                dma_x_part(sb, lo, 4)

            nc.sync.dma_start(out=bqk, in_=bqk_d[:, :])  # gates first bias-adds
            dma_w_pair0(wk0, wk_d, range(0, 1))   # tiny firsts: PE starts early
            dma_x_part(0, 0, 1)
            dma_w_pair0(wk0, wk_d, range(1, 4))
            dma_x_part(0, 1, 3)
            dma_w_pair0(wk0, wk_d, range(4, 8))
            dma_x_half(0, 4)
            dma_w_pair0(wq0, wq_d, range(0, 4))
            dma_x_half(1, 0)
            dma_w_pair0(wq0, wq_d, range(4, 8))
            dma_x_half(1, 4)
            nc.sync.dma_start(out=wv[:, 0:2048], in_=wv_d[:, 0:2048])
            nc.sync.dma_start(out=wv[:, 2048:4096], in_=wv_d[:, 2048:4096])
            nc.sync.dma_start(out=bv_b, in_=_bcast_ap(bv_d[:, :]))
            dma_x_half(2, 0)
            dma_x_half(2, 4)
            dma_x_half(3, 0)
            dma_x_half(3, 4)

            def dma_w_rest(dst, src_d):
                src = src_d[:, :].rearrange("p (c w) -> p c w", w=512)[:, :, 128:512]
                nc.sync.dma_start(out=dst.rearrange("p (c w) -> p c w", w=384),
                                  in_=src)

            dma_w_rest(wq2, wq_d)
            dma_w_rest(wk2, wk_d)
            nc.sync.dma_start(out=wo, in_=wo_d[:, :])
            nc.sync.dma_start(out=bo_b, in_=_bcast_ap(bo_d[:, :]))

            # ones columns of V_aug (col 64 of each 65-block)
            vones = vall.rearrange("p (x e) -> p x e", e=65)[:, :, 64:65]
            nc.vector.memset(vones, 1.0)

            # ---- emission helpers ----
            def xs(ch, sb):
                return xres[:, ch * S + sb * 512: ch * S + (sb + 1) * 512]

            def qk_unit(wmat, wcol0, dstT, bcol, p, sb):
                stride = wmat.shape[1] // 8
                ps_q = psP.tile([128, 512], F32, tag="pp", name="ps_q")
                for ch in range(8):
                    nc.tensor.matmul(
                        ps_q,
                        wmat[:, wcol0 + ch * stride: wcol0 + ch * stride + 128],
                        xs(ch, sb),
                        start=(ch == 0),
                        stop=(ch == 7),
                    )
                nc.vector.tensor_scalar_add(
                    dstT[:, p * S + sb * 512: p * S + (sb + 1) * 512],
                    ps_q,
                    bqk[:, bcol + p: bcol + p + 1],
                )

            def v_unit(sb, t4, ph):
                st = sb * 4 + t4
                ps_v = psP.tile([128, 256], F32, tag="pp", name="ps_v")
                for ch in range(8):
                    nc.tensor.matmul(
                        ps_v,
                        xs(ch, sb)[:, t4 * 128:(t4 + 1) * 128],
                        wv[:, ch * 512 + ph * 256: ch * 512 + (ph + 1) * 256],
                        start=(ch == 0),
                        stop=(ch == 7),
                    )
                dst = vall[:, st * 520 + ph * 260:
                           st * 520 + (ph + 1) * 260].rearrange(
                    "p (h e) -> p h e", e=65)[:, :, 0:64]
                nc.vector.tensor_add(
                    dst,
                    ps_v.rearrange("p (h e) -> p h e", e=64),
                    bv_b[:, ph * 256:(ph + 1) * 256].rearrange(
                        "p (h e) -> p h e", e=64),
                )

            oacc = {}  # (t, half) -> bf16 SBUF accumulator of first 2 pairs + bo/2

            def outproj_g1(t, half, pairs):
                ps_o = psP.tile([128, 512], F32, tag="pp", name="ps_o")
                for i, p in enumerate(pairs):
                    nc.tensor.matmul(
                        ps_o,
                        ctxT[:, p * S + t * 128: p * S + (t + 1) * 128],
                        wo[:, p * 1024 + half * 512: p * 1024 + (half + 1) * 512],
                        start=(i == 0),
                        stop=(i == 1),
                    )
                acc = oap.tile([128, 512], BF16, tag=f"oa{(t % 8) * 2 + half}",
                               name=f"oacc{t}_{half}")
                nc.vector.tensor_add(acc, ps_o,
                                     bo_b[:, half * 512:(half + 1) * 512])
                oacc[(t, half)] = acc

            ot_pend = {}  # t -> full-width ot tile; one DMA per t-chunk

            def outproj_g2(t, half, pairs, pool, tag, eng=None):
                ps_o = pool.tile([128, 512], F32, tag=tag, name="ps_o")
                for i, p in enumerate(pairs):
                    nc.tensor.matmul(
                        ps_o,
                        ctxT[:, p * S + t * 128: p * S + (t + 1) * 128],
                        wo[:, p * 1024 + half * 512: p * 1024 + (half + 1) * 512],
                        start=(i == 0),
                        stop=(i == 1),
                    )
                if t not in ot_pend:
                    ot_pend[t] = [otp.tile([128, 1024], BF16, tag="ot",
                                           name=f"ot{t}"), 0]
                ent = ot_pend[t]
                (eng or nc.vector).tensor_add(
                    ent[0][:, half * 512:(half + 1) * 512], ps_o,
                    oacc.pop((t, half)))
                ent[1] += 1
                if ent[1] == 2:
                    nc.sync.dma_start(out=out_d[t * 128:(t + 1) * 128, :],
                                      in_=ent[0])
                    del ot_pend[t]

            # ---- filler engine (cost in ~PE-ns at 2.4GHz) ----
            state = {"budget": 0.0, "blocks_done": 0}
            fillers = []   # list of (label, ready_after_blocks, cost, fn)

            def F(label, ready, cost, fn):
                fillers.append((label, ready, cost, fn))

            def emit_budget(add_ns, cap=2600.0):
                state["budget"] = min(state["budget"] + add_ns, cap)
                while fillers:
                    label, ready, cost, fn = fillers[0]
                    if ready > state["blocks_done"] or cost > state["budget"]:
                        break
                    fillers.pop(0)
                    fn()
                    state["budget"] -= cost

            def ensure(labels):
                """Emit all queued units whose label is in `labels`, skipping
                others (correctness gate: writers before readers).  Relative
                order within each label class is preserved; qk/v units and
                out-proj units are mutually independent so cross-class
                reordering is safe."""
                keep = []
                popped = False
                for f in fillers:
                    if f[0] in labels:
                        f[3]()
                        popped = True
                    else:
                        keep.append(f)
                fillers[:] = keep
                if popped:
                    state["budget"] = 0.0

            QK, VU, OP = 1710.0, 860.0, 440.0

            def mk_qk(wm, c0, dstT, bcol, p, sb):
                return lambda: qk_unit(wm, c0, dstT, bcol, p, sb)

            # queue in deadline order (interleaved qb0/qb1 block schedule)
            for sb in range(NSB):
                F("p1", 0, QK, mk_qk(wk2, 0, kT, 4, 1, sb))
            for sb in (0, 1):
                F("p1", 0, QK, mk_qk(wq2, 0, qT, 0, 1, sb))
            for sb in (2, 3):
                F("qx0", 0, QK, mk_qk(wq0, 0, qT, 0, 0, sb))
            vq = [("v", 0, VU, (lambda sb_, t4_: lambda: v_unit(sb_, t4_, 1))(sb, t4))
                  for sb in range(NSB) for t4 in range(4)]
            p2q = ([("p2", 0, QK, mk_qk(wk2, 128, kT, 4, 2, sb))
                    for sb in range(NSB)]
                   + [("p2", 0, QK, mk_qk(wq2, 128, qT, 0, 2, sb))
                      for sb in (0, 1)])
            while vq or p2q:
                for _ in range(3):
                    if vq:
                        fillers.append(vq.pop(0))
                if p2q:
                    fillers.append(p2q.pop(0))
            for sb in (2, 3):
                F("qx1", 0, QK, mk_qk(wq2, 0, qT, 0, 1, sb))
            for sb in range(NSB):
                F("p3", 0, QK, mk_qk(wk2, 256, kT, 4, 3, sb))
            for sb in (0, 1):
                F("p3", 0, QK, mk_qk(wq2, 256, qT, 0, 3, sb))
            for sb in (2, 3):
                F("qx2", 0, QK, mk_qk(wq2, 128, qT, 0, 2, sb))
            for t in range(8):
                for half in range(2):
                    F("g1a", 10, OP,
                      (lambda t_, h_: lambda: outproj_g1(t_, h_, (0, 1)))(
                          t, half))
            for sb in (2, 3):
                F("qx3", 0, QK, mk_qk(wq2, 256, qT, 0, 3, sb))
            for t in range(8, 16):
                for half in range(2):
                    F("g1b", 11, OP,
                      (lambda t_, h_: lambda: outproj_g1(t_, h_, (0, 1)))(
                          t, half))
            for t in range(8):
                for half in range(2):
                    F("g2a", 12, OP,
                      (lambda t_, h_: lambda: outproj_g2(
                          t_, h_, (2, 3), psP, "pp"))(t, half))

            # ---- attention machinery ----
            def scores_exp(h, qb, st):
                p = h // 2
                r0 = 64 * (h % 2)
                ps_s = psS.tile([128, 1024], F32, tag="ps")
                for half in range(2):
                    nc.tensor.matmul(
                        ps_s[:, half * 512:(half + 1) * 512],
                        kT[r0:r0 + 64,
                           p * S + st * 128: p * S + (st + 1) * 128],
                        qT[r0:r0 + 64,
                           p * S + qb * 1024 + half * 512:
                           p * S + qb * 1024 + (half + 1) * 512],
                        start=True,
                        stop=True,
                    )
                at = atp.tile([128, 1024], BF16, tag="at")
                nc.scalar.activation(at, ps_s, EXP, scale=0.125)
                return at

            def pv(h, st, at, pc):
                # PSUM start_tensor_calc zeroes the whole bank: only the first
                # write into each bank carries start=True; stop on the last.
                pcA, pcB = pc
                for c in range(8):
                    dst = (pcA if c < 4 else pcB)
                    nc.tensor.matmul(
                        dst[:, (c % 4) * 65:(c % 4) * 65 + 65],
                        at[:, c * 128:(c + 1) * 128],
                        vall[:, st * 520 + h * 65: st * 520 + (h + 1) * 65],
                        start=(st == 0 and c % 4 == 0),
                        stop=(st == NST - 1 and c % 4 == 3),
                        skip_group_check=True,
                    )

            nat_pool = {}

            def get_nat():
                return cn2.tile([128, 1024], BF16, tag="cnb", name="natb")

            def norm_side(pc_half, c0, hp, nat, rz, rcol):
                nc.vector.reciprocal(
                    rz[:, rcol:rcol + 4].rearrange("p (c o) -> p c o", o=1),
                    pc_half.rearrange("p (c e) -> p c e", e=65)[:, :, 64:65])
                for c in range(4):
                    nc.vector.tensor_scalar_mul(
                        nat[:, (c0 + c) * 128 + hp * 64:
                            (c0 + c) * 128 + (hp + 1) * 64],
                        pc_half[:, c * 65: c * 65 + 64],
                        rz[:, rcol + c:rcol + c + 1],
                    )

            def transpose_pair(p, qb, nat):
                # blockwise XBAR transpose: out[p, c, s] = nat[s, c*128 + p]
                nc.sync.dma_start_transpose(
                    out=ctxT[:, p * S + qb * 1024:
                             p * S + (qb + 1) * 1024].rearrange(
                        "p (c s) -> p c s", s=128),
                    in_=nat,
                )

            # ---- lead-in: pair0 k/q, h0 qb0 hand-interleaved with v-half0 ----
            qk_unit(wk0, 0, kT, 4, 0, 0)
            qk_unit(wq0, 0, qT, 0, 0, 0)
            qk_unit(wq0, 0, qT, 0, 0, 1)

            at_q = {}
            pc0 = (psC.tile([128, 260], F32, tag="pcA", name="pcA00"),
                   psC.tile([128, 260], F32, tag="pcB", name="pcB00"))
            nat_pool[0] = get_nat()
            for st in range(4):
                at_q[st] = scores_exp(0, 0, st)
            for t4 in range(4):
                v_unit(0, t4, 0)
            qk_unit(wk0, 0, kT, 4, 0, 1)
            for st in range(4, 8):
                at_q[st] = scores_exp(0, 0, st)
            for st in range(4):
                pv(0, st, at_q.pop(st), pc0)
            for t4 in range(4):
                v_unit(1, t4, 0)
            qk_unit(wk0, 0, kT, 4, 0, 2)
            for st in range(8, 12):
                at_q[st] = scores_exp(0, 0, st)
            for st in range(4, 8):
                pv(0, st, at_q.pop(st), pc0)
            qk_unit(wk0, 0, kT, 4, 0, 3)
            for st in range(12, 16):
                at_q[st] = scores_exp(0, 0, st)
            state["blocks_done"] = 1

            def pv0_tail(sts):
                def f():
                    for st in sts:
                        pv(0, st, at_q.pop(st), pc0)
                return f

            def norm0():
                rz0 = rzp.tile([128, 8], F32, tag="rz")
                norm_side(pc0[0], 0, 0, nat_pool[0], rz0, 0)
                norm_side(pc0[1], 4, 0, nat_pool[0], rz0, 4)

            lead_extra = [
                lambda: v_unit(2, 0, 0), lambda: v_unit(2, 1, 0),
                lambda: v_unit(2, 2, 0), lambda: v_unit(2, 3, 0),
                pv0_tail(range(8, 10)),
                lambda: v_unit(3, 0, 0), lambda: v_unit(3, 1, 0),
                pv0_tail(range(10, 12)),
                lambda: v_unit(3, 2, 0), lambda: v_unit(3, 3, 0),
                pv0_tail(range(12, 14)), pv0_tail(range(14, 16)),
                norm0, None, None, None,
            ]

            # ---- steady-state blocks ----
            # era A (qb0, PE-bound): drain deadline-pinned units aggressively;
            # era B (qb1, ACT-bound): consume the ready-gated op units.
            def block(h, qb, per_st, last=False, extra=None, pv_lag=3):
                p = h // 2
                # correctness gates (emission order = dependency order)
                gates = set()
                if h >= 2:
                    gates.add(f"p{p}")
                if qb == 1:
                    gates.add(f"qx{p}")
                if h >= 4:
                    gates.add("v")
                if gates:
                    ensure(gates)
                pc = (psC.tile([128, 260], F32, tag="pcA", name=f"pcA{h}{qb}"),
                      psC.tile([128, 260], F32, tag="pcB", name=f"pcB{h}{qb}"))
                if h % 2 == 0:
                    nat_pool[p] = get_nat()
                nat = nat_pool[p]
                at_pend = []
                emit_budget(per_st)
                for st in range(NST):
                    at = scores_exp(h, qb, st)
                    at_pend.append((st, at))
                    if extra and st < len(extra) and extra[st] is not None:
                        extra[st]()
                    else:
                        emit_budget(per_st)
                    if len(at_pend) >= pv_lag:
                        s0, a0 = at_pend.pop(0)
                        pv(h, s0, a0, pc)
                for s0, a0 in at_pend:
                    pv(h, s0, a0, pc)
                rz = rzp.tile([128, 8], F32, tag="rz")
                if not last:
                    norm_side(pc[0], 0, h % 2, nat, rz, 0)
                    norm_side(pc[1], 4, h % 2, nat, rz, 4)
                    if h % 2 == 1:
                        transpose_pair(p, qb, nat)
                else:
                    # tail: norm both halves, one blockwise transpose,
                    # then the pairs-2/3 completion units
                    norm_side(pc[0], 0, 1, nat, rz, 0)
                    norm_side(pc[1], 4, 1, nat, rz, 4)
                    transpose_pair(p, qb, nat)
                    for c in range(8):
                        for half in range(2):
                            outproj_g2(8 + c, half, (2, 3), psS, "ps")
                state["blocks_done"] += 1

            block(1, 0, 394.0, extra=lead_extra, pv_lag=13)
            seq = ((2, 0), (3, 0), (0, 1), (1, 1), (4, 0),
                   (5, 0), (2, 1), (3, 1), (6, 0), (7, 0), (4, 1),
                   (5, 1), (6, 1))
            for i, (h, qb) in enumerate(seq):
                block(h, qb, 500.0 if i >= 9 else 394.0)
            # drain anything still queued before the final block
            ensure({"p1", "p2", "p3", "v", "qx0", "qx1", "qx2", "qx3",
                    "g1a", "g2a", "g1b"})
            block(7, 1, 394.0, last=True)

    if legalize:
        legalize_waits(nc)
    return nc


def pack_core_inputs(c, x, Wq, bq, Wk, bk, Wv, bv, Wo, bo, S=S_FULL):
    """Pack full-model inputs into core c's device tensors (bf16)."""
    b = c // 2
    hh = c % 2
    hs = slice(hh * 8, hh * 8 + 8)
    bf = ml_dtypes.bfloat16

    def pack_w(W):  # [8, D, DH] -> [128, 4096]: free = chunk*512 + (h*64+dh)
        W2 = np.transpose(W, (1, 0, 2)).reshape(D, 512)      # [d, h*dh]
        return np.ascontiguousarray(
            np.transpose(W2.reshape(8, 128, 512), (1, 0, 2)).reshape(128, 4096)
        )

    xT = np.ascontiguousarray(x[b].T)                         # [D, S]
    wq = pack_w(Wq[hs])
    wk = pack_w(Wk[hs])
    wv = pack_w(Wv[hs])
    Wr = Wo[hh * 512:(hh + 1) * 512]
    wo = np.ascontiguousarray(
        np.transpose(Wr.reshape(4, 128, 1024), (1, 0, 2)).reshape(128, 4096)
    )
    bqk = np.concatenate(
        [bq[hs].reshape(4, 128).T, bk[hs].reshape(4, 128).T], axis=1
    )                                                         # [128, 8]
    bvp = bv[hs].reshape(1, 512)
    bop = (0.5 * bo).reshape(1, 1024)
    return {
        "xt": xT.astype(bf),
        "wq": wq.astype(bf),
        "wk": wk.astype(bf),
        "wv": wv.astype(bf),
        "wo": wo.astype(bf),
        "bqk": np.ascontiguousarray(bqk).astype(np.float32),
        "bv": bvp.astype(np.float32),
        "bo": bop.astype(np.float32),
    }


_NC_CACHE = {}


def _get_nc(S=S_FULL):
    if S not in _NC_CACHE:
        _NC_CACHE[S] = build_nc(S)
    return _NC_CACHE[S]


def kernel(x, Wq, bq, Wk, bk, Wv, bv, Wo, bo, _trace=False):
    x, Wq, bq, Wk, bk, Wv, bv, Wo, bo = (
        np.asarray(a, dtype=np.float32) for a in (x, Wq, bq, Wk, bk, Wv, bv, Wo, bo)
    )
    nc = _get_nc()
    in_maps = [
        pack_core_inputs(c, x, Wq, bq, Wk, bk, Wv, bv, Wo, bo) for c in range(NCORES)
    ]
    res = run_bass_kernel_spmd(nc, in_maps, list(range(NCORES)), trace=_trace)
    out = np.empty((B, S_FULL, D), dtype=np.float32)
    for b in range(B):
        out[b] = (res.results[2 * b]["out"].astype(np.float32)
                  + res.results[2 * b + 1]["out"].astype(np.float32))
    if _trace:
        kernel.last_results = res
    return out


# revision 53
# speedup vs baseline: 1.0048x; 1.0048x over previous
"""TRN2 Bass kernel for nn_MultiHeadAttention (B=4, S=2048, D=1024, H=16, DH=64).

Sharding (8 cores): core c -> batch b = c//2, head-half hh = c%2 (8 heads each).
Host sums the two per-core partial out-projections per batch.

All matmul operands are bf16 (1 cycle/row at any N, vs f32r's N>=256
requirement); accumulation stays f32 in PSUM.  Measured end-to-end rel err
~5.5e-3 vs the f32 reference (gate 2e-2).

Structure (single TileContext, one fluid phase; x resident in SBUF):
  - scores TRANSPOSED sT[sk, sq] = kT.T @ qT (stationary kT [64, 128-sk],
    moving qT [64, 512]); one 1024-wide exp per sk-tile -> at [128sk, 1024sq]
    bf16.
  - PV reoriented: at tile is STATIONARY [sk 128, sq 128-chunk], V_aug
    [sk 128, 65] is MOVING (N=65; full K=M=128) -> ctx accumulates in natural
    [sq, 65] layout, 65th column = softmax denominator on the sq partition.
    This halves PV's PE cycles vs streaming at with N=512 at M=65.
  - normalization: per-partition reciprocal + tensor_scalar_mul on DVE (no
    PE broadcast matmuls); each head pair shares one ctx_nat [sq 128, 1024]
    buffer per qb; a single blockwise XBAR DMA transpose (3D out AP:
    out[p,c,s] = in[s, c*128+p]) flips all 8 chunks into ctxT [128 feats, sq]
    in one DMA op (~620ns HWDGE overhead per op makes batching essential).
  - v-projection computes V_aug [sk, 65-per-head] with a memset-initialized
    ones column (no 4MB DMA); split into head-half units for fine scheduling.
  - out-projection splits into pairs-0/1 partials (SBUF bf16 accumulator,
    emitted early as filler) and pairs-2/3 completion units.
  - a budget-based filler engine interleaves projection/v/out-proj units into
    the attention stream (ACT needs 1038ns/sk-tile vs PE's 644ns) so PE and
    ACT both stay near-continuously busy; emission-order gates enforce that a
    block's pair projections are emitted before its scores (Tile dependency
    tracking is reader-after-writer in emission order).

This walrus build accepts only ONE sync-wait per instruction, so after
TileContext scheduling extra waits are split into single-wait NoOps on the
same engine (legalize_waits).
"""

import sys

if "/opt/trn_rl_repo" not in sys.path:
    sys.path.insert(0, "/opt/trn_rl_repo")

import numpy as np
import ml_dtypes

import concourse.bass as bass
import concourse.mybir as mybir
import concourse.tile as tile
from concourse.bass_utils import run_bass_kernel_spmd

F32 = mybir.dt.float32
BF16 = mybir.dt.bfloat16
EXP = mybir.ActivationFunctionType.Exp

B, S_FULL, D, H = 4, 2048, 1024, 16
DH = 64
NCORES = 8


def legalize_waits(nc, max_waits=1):
    """Split >max_waits sync-waits per instruction into single-wait NoOps on
    the same engine, placed immediately before (per-engine order preserved)."""
    n = 0
    for fn in nc.m.functions:
        for blk in fn.blocks:
            out = []
            for inst in blk.instructions:
                si = inst.sync_info
                if si is not None and len(si.on_wait) > max_waits:
                    waits = list(si.on_wait)
                    for w in waits[:-max_waits]:
                        nop = mybir.InstNoOp(
                            name=f"WSPLIT-{n}", ins=[], outs=[], engine=inst.engine
                        )
                        n += 1
                        nop.sync_info = mybir.SyncInfo(on_wait=[w], on_update=[])
                        out.append(nop)
                    inst.sync_info = mybir.SyncInfo(
                        on_wait=waits[-max_waits:], on_update=list(si.on_update)
                    )
                out.append(inst)
            blk.instructions[:] = out
    return n


def _bcast_ap(src_ap, parts=128):
    """Partition-broadcast a [1, N] AP to [parts, N] via a step-0 dim."""
    return bass.AP(
        tensor=src_ap.tensor,
        offset=src_ap.offset,
        ap=[[0, parts], list(src_ap.ap[-1])],
    )


def build_nc(S=S_FULL, legalize=True):
    NST = S // 128   # sk tiles
    NSB = S // 512   # 512-wide s blocks (projection granularity)
    nc = bass.Bass()
    xT_d = nc.dram_tensor("xt", [D, S], BF16, kind="ExternalInput")
    wq_d = nc.dram_tensor("wq", [128, 4096], BF16, kind="ExternalInput")
    wk_d = nc.dram_tensor("wk", [128, 4096], BF16, kind="ExternalInput")
    wv_d = nc.dram_tensor("wv", [128, 4096], BF16, kind="ExternalInput")
    wo_d = nc.dram_tensor("wo", [128, 4096], BF16, kind="ExternalInput")
    bqk_d = nc.dram_tensor("bqk", [128, 8], F32, kind="ExternalInput")
    bv_d = nc.dram_tensor("bv", [1, 512], F32, kind="ExternalInput")
    bo_d = nc.dram_tensor("bo", [1, 1024], F32, kind="ExternalInput")
    out_d = nc.dram_tensor("out", [S, 1024], BF16, kind="ExternalOutput")

    with tile.TileContext(nc) as tc, nc.allow_low_precision(
        reason="bf16 matmul operands are intentional; f32 accumulate"
    ):
        with tc.tile_pool(name="persist", bufs=1) as pp, \
             tc.tile_pool(name="psS", bufs=2, space="PSUM") as psS, \
             tc.tile_pool(name="psC", bufs=1, space="PSUM") as psC, \
             tc.tile_pool(name="psP", bufs=2, space="PSUM") as psP, \
             tc.tile_pool(name="atp", bufs=13) as atp, \
             tc.tile_pool(name="cn2", bufs=2) as cn2, \
             tc.tile_pool(name="rzp", bufs=4) as rzp, \
             tc.tile_pool(name="oap", bufs=2) as oap, \
             tc.tile_pool(name="otp", bufs=3) as otp:
            xres = pp.tile([128, 8 * S], BF16)          # d-chunk ch at cols ch*S
            qT = pp.tile([128, 4 * S], BF16)
            kT = pp.tile([128, 4 * S], BF16)
            vall = pp.tile([128, NST * 520], BF16)      # per st: 8 heads x 65
            ctxT = pp.tile([128, 4 * S], BF16)
            wq0 = pp.tile([128, 1024], BF16)
            wk0 = pp.tile([128, 1024], BF16)
            wq2 = pp.tile([128, 3072], BF16)
            wk2 = pp.tile([128, 3072], BF16)
            wv = pp.tile([128, 4096], BF16)
            wo = pp.tile([128, 4096], BF16)
            bqk = pp.tile([128, 8], F32)
            bv_b = pp.tile([128, 512], F32)
            bo_b = pp.tile([128, 1024], F32)

            # ---- DMA issue order: first-needed bytes first ----
            def dma_w_pair0(dst, src_d, chs):
                src = src_d[:, chs[0] * 512: chs[-1] * 512 + 512].rearrange(
                    "p (c w) -> p c w", w=512)[:, :, 0:128]
                nc.sync.dma_start(
                    out=dst[:, chs[0] * 128: chs[-1] * 128 + 128].rearrange(
                        "p (c w) -> p c w", w=128),
                    in_=src)

            def dma_x_part(sb, lo, n):
                # d-chunks lo..lo+n-1 of s-block sb in one DMA
                dst = xres[:, lo * S:(lo + n) * S].rearrange(
                    "p (c s) -> p c s", c=n)[:, :, sb * 512:(sb + 1) * 512]
                src = xT_d[lo * 128:(lo + n) * 128,
                           sb * 512:(sb + 1) * 512].rearrange(
                    "(c p) s -> p c s", c=n)
                nc.sync.dma_start(out=dst, in_=src)

            def dma_x_half(sb, lo):
                dma_x_part(sb, lo, 4)

            nc.sync.dma_start(out=bqk, in_=bqk_d[:, :])  # gates first bias-adds
            dma_w_pair0(wk0, wk_d, range(0, 1))   # tiny firsts: PE starts early
            dma_x_part(0, 0, 1)
            dma_w_pair0(wk0, wk_d, range(1, 4))
            dma_x_part(0, 1, 3)
            dma_w_pair0(wk0, wk_d, range(4, 8))
            dma_x_half(0, 4)
            dma_w_pair0(wq0, wq_d, range(0, 4))
            dma_x_half(1, 0)
            dma_w_pair0(wq0, wq_d, range(4, 8))
            dma_x_half(1, 4)
            nc.sync.dma_start(out=wv[:, 0:2048], in_=wv_d[:, 0:2048])
            nc.sync.dma_start(out=wv[:, 2048:4096], in_=wv_d[:, 2048:4096])
            nc.sync.dma_start(out=bv_b, in_=_bcast_ap(bv_d[:, :]))
            dma_x_half(2, 0)
            dma_x_half(2, 4)
            dma_x_half(3, 0)
            dma_x_half(3, 4)

            def dma_w_rest(dst, src_d):
                src = src_d[:, :].rearrange("p (c w) -> p c w", w=512)[:, :, 128:512]
                nc.sync.dma_start(out=dst.rearrange("p (c w) -> p c w", w=384),
                                  in_=src)

            dma_w_rest(wq2, wq_d)
            dma_w_rest(wk2, wk_d)
            nc.sync.dma_start(out=wo, in_=wo_d[:, :])
            nc.sync.dma_start(out=bo_b, in_=_bcast_ap(bo_d[:, :]))

            # ones columns of V_aug (col 64 of each 65-block)
            vones = vall.rearrange("p (x e) -> p x e", e=65)[:, :, 64:65]
            nc.vector.memset(vones, 1.0)

            # ---- emission helpers ----
            def xs(ch, sb):
                return xres[:, ch * S + sb * 512: ch * S + (sb + 1) * 512]

            def qk_unit(wmat, wcol0, dstT, bcol, p, sb):
                stride = wmat.shape[1] // 8
                ps_q = psP.tile([128, 512], F32, tag="pp", name="ps_q")
                for ch in range(8):
                    nc.tensor.matmul(
                        ps_q,
                        wmat[:, wcol0 + ch * stride: wcol0 + ch * stride + 128],
                        xs(ch, sb),
                        start=(ch == 0),
                        stop=(ch == 7),
                    )
                nc.vector.tensor_scalar_add(
                    dstT[:, p * S + sb * 512: p * S + (sb + 1) * 512],
                    ps_q,
                    bqk[:, bcol + p: bcol + p + 1],
                )

            def v_unit(sb, t4, ph):
                st = sb * 4 + t4
                ps_v = psP.tile([128, 256], F32, tag="pp", name="ps_v")
                for ch in range(8):
                    nc.tensor.matmul(
                        ps_v,
                        xs(ch, sb)[:, t4 * 128:(t4 + 1) * 128],
                        wv[:, ch * 512 + ph * 256: ch * 512 + (ph + 1) * 256],
                        start=(ch == 0),
                        stop=(ch == 7),
                    )
                dst = vall[:, st * 520 + ph * 260:
                           st * 520 + (ph + 1) * 260].rearrange(
                    "p (h e) -> p h e", e=65)[:, :, 0:64]
                nc.vector.tensor_add(
                    dst,
                    ps_v.rearrange("p (h e) -> p h e", e=64),
                    bv_b[:, ph * 256:(ph + 1) * 256].rearrange(
                        "p (h e) -> p h e", e=64),
                )

            oacc = {}  # (t, half) -> bf16 SBUF accumulator of first 2 pairs + bo/2

            def outproj_g1(t, half, pairs):
                ps_o = psP.tile([128, 512], F32, tag="pp", name="ps_o")
                for i, p in enumerate(pairs):
                    nc.tensor.matmul(
                        ps_o,
                        ctxT[:, p * S + t * 128: p * S + (t + 1) * 128],
                        wo[:, p * 1024 + half * 512: p * 1024 + (half + 1) * 512],
                        start=(i == 0),
                        stop=(i == 1),
                    )
                if t >= 8:
                    # qb1 accumulators are full-width: the tail then needs
                    # only one DVE add + one DMA per token tile
                    if t not in oacc:
                        oacc[t] = oap.tile([128, 1024], BF16, tag=f"ob{t - 8}",
                                           name=f"oaccb{t}", bufs=1)
                    nc.vector.tensor_add(
                        oacc[t][:, half * 512:(half + 1) * 512], ps_o,
                        bo_b[:, half * 512:(half + 1) * 512])
                else:
                    acc = oap.tile([128, 512], BF16,
                                   tag=f"oa{(t % 8) * 2 + half}",
                                   name=f"oacc{t}_{half}", bufs=1)
                    nc.vector.tensor_add(acc, ps_o,
                                         bo_b[:, half * 512:(half + 1) * 512])
                    oacc[(t, half)] = acc

            ot_pend = {}  # t -> full-width ot tile; one DMA per t-chunk

            def outproj_g2(t, half, pairs, pool, tag, eng=None):
                ps_o = pool.tile([128, 512], F32, tag=tag, name="ps_o")
                for i, p in enumerate(pairs):
                    nc.tensor.matmul(
                        ps_o,
                        ctxT[:, p * S + t * 128: p * S + (t + 1) * 128],
                        wo[:, p * 1024 + half * 512: p * 1024 + (half + 1) * 512],
                        start=(i == 0),
                        stop=(i == 1),
                    )
                if t not in ot_pend:
                    ot_pend[t] = [otp.tile([128, 1024], BF16, tag="ot",
                                           name=f"ot{t}"), 0]
                ent = ot_pend[t]
                (eng or nc.vector).tensor_add(
                    ent[0][:, half * 512:(half + 1) * 512], ps_o,
                    oacc.pop((t, half)))
                ent[1] += 1
                if ent[1] == 2:
                    nc.sync.dma_start(out=out_d[t * 128:(t + 1) * 128, :],
                                      in_=ent[0])
                    del ot_pend[t]

            # ---- filler engine (cost in ~PE-ns at 2.4GHz) ----
            state = {"budget": 0.0, "blocks_done": 0}
            fillers = []   # list of (label, ready_after_blocks, cost, fn)

            def F(label, ready, cost, fn):
                fillers.append((label, ready, cost, fn))

            def emit_budget(add_ns, cap=2600.0):
                state["budget"] = min(state["budget"] + add_ns, cap)
                while fillers:
                    label, ready, cost, fn = fillers[0]
                    if ready > state["blocks_done"] or cost > state["budget"]:
                        break
                    fillers.pop(0)
                    fn()
                    state["budget"] -= cost

            def ensure(labels):
                """Emit all queued units whose label is in `labels`, skipping
                others (correctness gate: writers before readers).  Relative
                order within each label class is preserved; qk/v units and
                out-proj units are mutually independent so cross-class
                reordering is safe."""
                keep = []
                popped = False
                for f in fillers:
                    if f[0] in labels:
                        f[3]()
                        popped = True
                    else:
                        keep.append(f)
                fillers[:] = keep
                if popped:
                    state["budget"] = 0.0

            QK, VU, OP = 1710.0, 860.0, 440.0

            def mk_qk(wm, c0, dstT, bcol, p, sb):
                return lambda: qk_unit(wm, c0, dstT, bcol, p, sb)

            # queue in deadline order (interleaved qb0/qb1 block schedule)
            for sb in range(NSB):
                F("p1", 0, QK, mk_qk(wk2, 0, kT, 4, 1, sb))
            for sb in (0, 1):
                F("p1", 0, QK, mk_qk(wq2, 0, qT, 0, 1, sb))
            for sb in (2, 3):
                F("qx0", 0, QK, mk_qk(wq0, 0, qT, 0, 0, sb))
            vq = [("v", 0, VU, (lambda sb_, t4_: lambda: v_unit(sb_, t4_, 1))(sb, t4))
                  for sb in range(NSB) for t4 in range(4)]
            p2q = ([("p2", 0, QK, mk_qk(wk2, 128, kT, 4, 2, sb))
                    for sb in range(NSB)]
                   + [("p2", 0, QK, mk_qk(wq2, 128, qT, 0, 2, sb))
                      for sb in (0, 1)])
            while vq or p2q:
                for _ in range(3):
                    if vq:
                        fillers.append(vq.pop(0))
                if p2q:
                    fillers.append(p2q.pop(0))
            for sb in (2, 3):
                F("qx1", 0, QK, mk_qk(wq2, 0, qT, 0, 1, sb))
            for sb in range(NSB):
                F("p3", 0, QK, mk_qk(wk2, 256, kT, 4, 3, sb))
            for sb in (0, 1):
                F("p3", 0, QK, mk_qk(wq2, 256, qT, 0, 3, sb))
            for sb in (2, 3):
                F("qx2", 0, QK, mk_qk(wq2, 128, qT, 0, 2, sb))
            for t in range(8):
                for half in range(2):
                    F("g1a", 10, OP,
                      (lambda t_, h_: lambda: outproj_g1(t_, h_, (0, 1)))(
                          t, half))
            for sb in (2, 3):
                F("qx3", 0, QK, mk_qk(wq2, 256, qT, 0, 3, sb))
            for t in range(8, 16):
                for half in range(2):
                    F("g1b", 11, OP,
                      (lambda t_, h_: lambda: outproj_g1(t_, h_, (0, 1)))(
                          t, half))
            for t in range(8):
                for half in range(2):
                    F("g2a", 12, OP,
                      (lambda t_, h_: lambda: outproj_g2(
                          t_, h_, (2, 3), psP, "pp"))(t, half))

            def outproj_g2_full(t):
                ps_o = psS.tile([128, 1024], F32, tag="ps", name="ps_of")
                for i, p in enumerate((2, 3)):
                    for half in range(2):
                        nc.tensor.matmul(
                            ps_o[:, half * 512:(half + 1) * 512],
                            ctxT[:, p * S + t * 128: p * S + (t + 1) * 128],
                            wo[:, p * 1024 + half * 512:
                               p * 1024 + (half + 1) * 512],
                            start=(i == 0),
                            stop=(i == 1),
                        )
                ot = otp.tile([128, 1024], BF16, tag="ot", name=f"otf{t}")
                nc.vector.tensor_add(ot, ps_o, oacc.pop(t))
                nc.sync.dma_start(out=out_d[t * 128:(t + 1) * 128, :], in_=ot)

            # ---- attention machinery ----
            def scores_exp(h, qb, st):
                p = h // 2
                r0 = 64 * (h % 2)
                ps_s = psS.tile([128, 1024], F32, tag="ps")
                for half in range(2):
                    nc.tensor.matmul(
                        ps_s[:, half * 512:(half + 1) * 512],
                        kT[r0:r0 + 64,
                           p * S + st * 128: p * S + (st + 1) * 128],
                        qT[r0:r0 + 64,
                           p * S + qb * 1024 + half * 512:
                           p * S + qb * 1024 + (half + 1) * 512],
                        start=True,
                        stop=True,
                    )
                at = atp.tile([128, 1024], BF16, tag="at")
                nc.scalar.activation(at, ps_s, EXP, scale=0.125)
                return at

            def pv(h, st, at, pc):
                # PSUM start_tensor_calc zeroes the whole bank: only the first
                # write into each bank carries start=True; stop on the last.
                pcA, pcB = pc
                for c in range(8):
                    dst = (pcA if c < 4 else pcB)
                    nc.tensor.matmul(
                        dst[:, (c % 4) * 65:(c % 4) * 65 + 65],
                        at[:, c * 128:(c + 1) * 128],
                        vall[:, st * 520 + h * 65: st * 520 + (h + 1) * 65],
                        start=(st == 0 and c % 4 == 0),
                        stop=(st == NST - 1 and c % 4 == 3),
                        skip_group_check=True,
                    )

            nat_pool = {}

            def get_nat():
                return cn2.tile([128, 1024], BF16, tag="cnb", name="natb")

            def norm_side(pc_half, c0, hp, nat, rz, rcol):
                nc.vector.reciprocal(
                    rz[:, rcol:rcol + 4].rearrange("p (c o) -> p c o", o=1),
                    pc_half.rearrange("p (c e) -> p c e", e=65)[:, :, 64:65])
                for c in range(4):
                    nc.vector.tensor_scalar_mul(
                        nat[:, (c0 + c) * 128 + hp * 64:
                            (c0 + c) * 128 + (hp + 1) * 64],
                        pc_half[:, c * 65: c * 65 + 64],
                        rz[:, rcol + c:rcol + c + 1],
                    )

            def transpose_pair(p, qb, nat):
                # blockwise XBAR transpose: out[p, c, s] = nat[s, c*128 + p]
                nc.sync.dma_start_transpose(
                    out=ctxT[:, p * S + qb * 1024:
                             p * S + (qb + 1) * 1024].rearrange(
                        "p (c s) -> p c s", s=128),
                    in_=nat,
                )

            # ---- lead-in: pair0 k/q, h0 qb0 hand-interleaved with v-half0 ----
            qk_unit(wk0, 0, kT, 4, 0, 0)
            qk_unit(wq0, 0, qT, 0, 0, 0)
            qk_unit(wq0, 0, qT, 0, 0, 1)

            at_q = {}
            pc0 = (psC.tile([128, 260], F32, tag="pcA", name="pcA00"),
                   psC.tile([128, 260], F32, tag="pcB", name="pcB00"))
            nat_pool[0] = get_nat()
            for st in range(4):
                at_q[st] = scores_exp(0, 0, st)
            for t4 in range(4):
                v_unit(0, t4, 0)
            qk_unit(wk0, 0, kT, 4, 0, 1)
            for st in range(4, 8):
                at_q[st] = scores_exp(0, 0, st)
            for st in range(4):
                pv(0, st, at_q.pop(st), pc0)
            for t4 in range(4):
                v_unit(1, t4, 0)
            qk_unit(wk0, 0, kT, 4, 0, 2)
            for st in range(8, 12):
                at_q[st] = scores_exp(0, 0, st)
            for st in range(4, 8):
                pv(0, st, at_q.pop(st), pc0)
            qk_unit(wk0, 0, kT, 4, 0, 3)
            for st in range(12, 16):
                at_q[st] = scores_exp(0, 0, st)
            state["blocks_done"] = 1

            def pv0_tail(sts):
                def f():
                    for st in sts:
                        pv(0, st, at_q.pop(st), pc0)
                return f

            def norm0():
                rz0 = rzp.tile([128, 8], F32, tag="rz")
                norm_side(pc0[0], 0, 0, nat_pool[0], rz0, 0)
                norm_side(pc0[1], 4, 0, nat_pool[0], rz0, 4)

            lead_extra = [
                lambda: v_unit(2, 0, 0), lambda: v_unit(2, 1, 0),
                lambda: v_unit(2, 2, 0), lambda: v_unit(2, 3, 0),
                pv0_tail(range(8, 10)),
                lambda: v_unit(3, 0, 0), lambda: v_unit(3, 1, 0),
                pv0_tail(range(10, 12)),
                lambda: v_unit(3, 2, 0), lambda: v_unit(3, 3, 0),
                pv0_tail(range(12, 14)), pv0_tail(range(14, 16)),
                norm0, None, None, None,
            ]

            # ---- steady-state blocks ----
            # era A (qb0, PE-bound): drain deadline-pinned units aggressively;
            # era B (qb1, ACT-bound): consume the ready-gated op units.
            def block(h, qb, per_st, last=False, extra=None, pv_lag=3):
                p = h // 2
                # correctness gates (emission order = dependency order)
                gates = set()
                if h >= 2:
                    gates.add(f"p{p}")
                if qb == 1:
                    gates.add(f"qx{p}")
                if h >= 4:
                    gates.add("v")
                if gates:
                    ensure(gates)
                pc = (psC.tile([128, 260], F32, tag="pcA", name=f"pcA{h}{qb}"),
                      psC.tile([128, 260], F32, tag="pcB", name=f"pcB{h}{qb}"))
                if h % 2 == 0:
                    nat_pool[p] = get_nat()
                nat = nat_pool[p]
                at_pend = []
                emit_budget(per_st)
                for st in range(NST):
                    at = scores_exp(h, qb, st)
                    at_pend.append((st, at))
                    if extra and st < len(extra) and extra[st] is not None:
                        extra[st]()
                    else:
                        emit_budget(per_st)
                    if len(at_pend) >= pv_lag:
                        s0, a0 = at_pend.pop(0)
                        pv(h, s0, a0, pc)
                for s0, a0 in at_pend:
                    pv(h, s0, a0, pc)
                rz = rzp.tile([128, 8], F32, tag="rz")
                if not last:
                    norm_side(pc[0], 0, h % 2, nat, rz, 0)
                    norm_side(pc[1], 4, h % 2, nat, rz, 4)
                    if h % 2 == 1:
                        transpose_pair(p, qb, nat)
                else:
                    # tail: norm both halves, one blockwise transpose,
                    # then the pairs-2/3 completion units
                    norm_side(pc[0], 0, 1, nat, rz, 0)
                    norm_side(pc[1], 4, 1, nat, rz, 4)
                    transpose_pair(p, qb, nat)
                    for c in range(8):
                        outproj_g2_full(8 + c)
                state["blocks_done"] += 1

            block(1, 0, 394.0, extra=lead_extra, pv_lag=13)
            seq = ((2, 0), (3, 0), (0, 1), (1, 1), (4, 0),
                   (5, 0), (2, 1), (3, 1), (6, 0), (7, 0), (4, 1),
                   (5, 1), (6, 1))
            for i, (h, qb) in enumerate(seq):
                block(h, qb, 500.0 if i >= 9 else 394.0)
            # drain anything still queued before the final block
            ensure({"p1", "p2", "p3", "v", "qx0", "qx1", "qx2", "qx3",
                    "g1a", "g2a", "g1b"})
            block(7, 1, 394.0, last=True)

    if legalize:
        legalize_waits(nc)
    return nc


def pack_core_inputs(c, x, Wq, bq, Wk, bk, Wv, bv, Wo, bo, S=S_FULL):
    """Pack full-model inputs into core c's device tensors (bf16)."""
    b = c // 2
    hh = c % 2
    hs = slice(hh * 8, hh * 8 + 8)
    bf = ml_dtypes.bfloat16

    def pack_w(W):  # [8, D, DH] -> [128, 4096]: free = chunk*512 + (h*64+dh)
        W2 = np.transpose(W, (1, 0, 2)).reshape(D, 512)      # [d, h*dh]
        return np.ascontiguousarray(
            np.transpose(W2.reshape(8, 128, 512), (1, 0, 2)).reshape(128, 4096)
        )

    xT = np.ascontiguousarray(x[b].T)                         # [D, S]
    wq = pack_w(Wq[hs])
    wk = pack_w(Wk[hs])
    wv = pack_w(Wv[hs])
    Wr = Wo[hh * 512:(hh + 1) * 512]
    wo = np.ascontiguousarray(
        np.transpose(Wr.reshape(4, 128, 1024), (1, 0, 2)).reshape(128, 4096)
    )
    bqk = np.concatenate(
        [bq[hs].reshape(4, 128).T, bk[hs].reshape(4, 128).T], axis=1
    )                                                         # [128, 8]
    bvp = bv[hs].reshape(1, 512)
    bop = (0.5 * bo).reshape(1, 1024)
    return {
        "xt": xT.astype(bf),
        "wq": wq.astype(bf),
        "wk": wk.astype(bf),
        "wv": wv.astype(bf),
        "wo": wo.astype(bf),
        "bqk": np.ascontiguousarray(bqk).astype(np.float32),
        "bv": bvp.astype(np.float32),
        "bo": bop.astype(np.float32),
    }


_NC_CACHE = {}


def _get_nc(S=S_FULL):
    if S not in _NC_CACHE:
        _NC_CACHE[S] = build_nc(S)
    return _NC_CACHE[S]


def kernel(x, Wq, bq, Wk, bk, Wv, bv, Wo, bo, _trace=False):
    x, Wq, bq, Wk, bk, Wv, bv, Wo, bo = (
        np.asarray(a, dtype=np.float32) for a in (x, Wq, bq, Wk, bk, Wv, bv, Wo, bo)
    )
    nc = _get_nc()
    in_maps = [
        pack_core_inputs(c, x, Wq, bq, Wk, bk, Wv, bv, Wo, bo) for c in range(NCORES)
    ]
    res = run_bass_kernel_spmd(nc, in_maps, list(range(NCORES)), trace=_trace)
    out = np.empty((B, S_FULL, D), dtype=np.float32)
    for b in range(B):
        out[b] = (res.results[2 * b]["out"].astype(np.float32)
                  + res.results[2 * b + 1]["out"].astype(np.float32))
    if _trace:
        kernel.last_results = res
    return out
